# revision 26
# baseline (speedup 1.0000x reference)
"""Trainium2 Bass kernel for nn_EquivariantConvExp.

Model: 4 blocks of { z = conv_exp(z, k_i); z = tanh(z) } on [B=256,1,256,256],
where conv_exp is the 10-term truncated power series of a 7x7 same-padded
single-channel conv. The log_det output is input-independent (host-computed).

Strategy: pure data parallel, 32 samples per core (16 pairs of 2 samples).
Per-sample conv term K@q is computed as banded matmuls along H:
  out[h_out, w] += Band_kc.T @ q[h_chunk, w + kc - 3]      (kc = 0..6)
with the image stored [h (2 chunks x 128 partitions), sample(2), 256+6 pad].
Cross-h-chunk band contributions come from 2 small fixup matmuls whose rhs is
a [21, 2, 256] gather (3 boundary rows x 7 w-shifts) built by SBUF->SBUF DMAs
with overlapping-window access patterns. The series sum accumulates in PSUM
via scaled-identity matmuls; the product chain is kept pre-divided
(q_n = prod_n/(n+1)) so accumulation is a plain matmul accumulate. Matmuls
run as float32r: full fp32 bits at bf16-rate for moving dim >= 256.

Raw bass (no Tile): this walrus build encodes at most ~1 semaphore wait per
instruction, so synchronization uses standalone wait_ge instructions with
hand-maintained cumulative counters per engine.
"""

import sys
from contextlib import ExitStack

import numpy as np

sys.path.insert(0, "/opt/trn_rl_repo")

import concourse.bass as bass
from concourse import mybir
from concourse.bass_utils import run_bass_kernel_spmd

F32 = mybir.dt.float32
F32R = mybir.dt.float32r
Tanh = mybir.ActivationFunctionType.Tanh

N_BLOCKS = 4
N_TERMS = 10
KS = 7
H = W = 256
N_CORES = 8
B = 256
PER_CORE = B // N_CORES          # 32
N_PAIRS = PER_CORE // 2          # 16
WP = W + 6                       # padded width 262
QF = 2 * WP                      # q free elems per hc: s(2) * 262
QROW = 2 * QF                    # q free elems per partition: hc*s*262


def _host_matrices(filters):
    """Banded / fixup / scaled-identity matrices, laid out for SBUF."""
    ks = [np.asarray(filters[fi, 0, 0], np.float32) for fi in reversed(range(N_BLOCKS))]
    bands = np.zeros((128, N_BLOCKS, KS, 128), np.float32)   # [p_in, blk, kc, p_out]
    fups = np.zeros((21, N_BLOCKS, 2, 128), np.float32)      # [(j*7+kc), blk, dir, m]
    for b, k in enumerate(ks):
        for kc in range(KS):
            for pi in range(128):
                for po in range(max(0, pi - 3), min(128, pi + 4)):
                    bands[pi, b, kc, po] = k[pi - po + 3, kc]
        for j in range(3):
            for kc in range(KS):
                rr = j * 7 + kc
                for m in range(128):
                    dh = (128 + j) - m            # dir 0 (down): h_in=128+j, h_out=m
                    if abs(dh) <= 3:
                        fups[rr, b, 0, m] = k[dh + 3, kc]
                    dh = (125 + j) - (128 + m)    # dir 1 (up): h_in=125+j, h_out=128+m
                    if abs(dh) <= 3:
                        fups[rr, b, 1, m] = k[dh + 3, kc]
    return bands, fups


def _build_program(n_pairs):
    nc = bass.Bass()
    x_in = nc.declare_dram_parameter("x", [2 * n_pairs, H, W], F32R, isOutput=False)
    bands_in = nc.declare_dram_parameter("bands", [128, N_BLOCKS, KS, 128], F32R, isOutput=False)
    fups_in = nc.declare_dram_parameter("fups", [21, N_BLOCKS, 2, 128], F32R, isOutput=False)
    z_out = nc.declare_dram_parameter("z_out", [2 * n_pairs, H, W], F32R, isOutput=True)

    r = lambda ap: ap
    ctx = ExitStack()
    sb = lambda name, shape, dt=F32R: ctx.enter_context(nc.sbuf_tensor(name, shape, dt))
    ps = lambda name, shape: ctx.enter_context(nc.psum_tensor(name, shape, F32))
    sem = lambda name: ctx.enter_context(nc.semaphore(name))
    Mul, Add = mybir.AluOpType.mult, mybir.AluOpType.add

    with ctx:
        bands_sb = sb("bands_sb", [128, N_BLOCKS, KS, 128])
        fups_sb = sb("fups_sb", [21, N_BLOCKS, 2, 128])
        qa = [sb(f"qa{sl}", [128, 2, 2, WP]) for sl in range(2)]
        qb = [sb(f"qb{sl}", [128, 2, 2, WP]) for sl in range(2)]
        gt = [[sb(f"g{sl}{d}", [21, 2, W]) for d in range(2)] for sl in range(2)]
        prod = [[ps(f"pr{sl}{hc}", [128, 2, W]) for hc in range(2)] for sl in range(2)]
        resb = [sb(f"res{sl}", [128, 2, 2, W], F32) for sl in range(2)]

        s_w = sem("s_w")
        s_init = sem("s_init")
        s_pe = [sem(f"s_pe{sl}") for sl in range(2)]
        s_dve = [sem(f"s_dve{sl}") for sl in range(2)]
        s_act = [sem(f"s_act{sl}") for sl in range(2)]
        s_g = [sem(f"s_g{sl}") for sl in range(2)]
        s_in = [sem(f"s_in{sl}") for sl in range(2)]
        s_out = [sem(f"s_out{sl}") for sl in range(2)]

        def slots(pg):
            return [sl for sl in range(2) if 2 * pg + sl < n_pairs]

        def qsrc(sl, n):
            if n == 0:
                return qa[sl]
            return qb[sl] if n % 2 == 1 else qa[sl]

        # per-slot cumulative milestone formulas (pp = pair index on slot)
        NT, NB = N_TERMS, N_BLOCKS
        P = lambda pp, b, n: (pp * NB + b) * NT + n          # s_pe after fixup_n
        D = lambda pp, b, n: (pp * NB + b) * NT + n          # s_dve after pkg_n
        A = lambda pp, b, n: (pp * NB + b) * NT + n          # s_act after drain_n (n<NT)
        AT = lambda pp, b: (pp * NB + b) * NT + NT           # s_act after tanh
        GV = lambda pp, b, k: ((pp * NB + b) * NT + k + 1) * 64  # s_g after round k

        n_pg = (n_pairs + 1) // 2

        with nc.Block() as block:

            @block.sync
            def _(sync):
                sync.dma_start(bands_sb[:], bands_in[:]).then_inc(s_w, 16)
                sync.dma_start(fups_sb[:], fups_in[:]).then_inc(s_w, 16)

                def load_pair(sl, pg):
                    p = 2 * pg + sl
                    pp = pg
                    if pp == 0:
                        sync.wait_ge(s_init, 1)
                    else:
                        sync.wait_ge(s_out[sl], pp * 64)
                    for hc in range(2):
                        for s in range(2):
                            sync.dma_start(
                                qa[sl][:, hc, s, 3:3 + W],
                                x_in[2 * p + s, 128 * hc:128 * (hc + 1), :],
                            ).then_inc(s_in[sl], 16)

                def ground(sl, pp, b, k):
                    # gather for term k+1's fixups, reading q_k
                    qt = qsrc(sl, k)
                    t = qt[:].tensor
                    if k == 0:
                        if b == 0:
                            sync.wait_ge(s_in[sl], (pp + 1) * 64)
                        else:
                            sync.wait_ge(s_act[sl], AT(pp, b - 1))
                        # dir1 reads hc0 (chunk0): same source readiness
                    else:
                        sync.wait_ge(s_act[sl], A(pp, b, k))
                    for s in range(2):
                        src = bass.AP(t, QF + s * WP, [[QROW, 3], [1, KS], [1, W]])
                        sync.dma_start(gt[sl][0][:, s, :], src).then_inc(s_g[sl], 16)
                    if k != 0:
                        sync.wait_ge(s_dve[sl], D(pp, b, k))
                    for s in range(2):
                        src = bass.AP(t, 125 * QROW + s * WP, [[QROW, 3], [1, KS], [1, W]])
                        sync.dma_start(gt[sl][1][:, s, :], src).then_inc(s_g[sl], 16)

                def store_pair(sl, pg):
                    p = 2 * pg + sl
                    sync.wait_ge(s_act[sl], AT(pg, N_BLOCKS - 1))
                    for hc in range(2):
                        for s in range(2):
                            sync.dma_start(
                                z_out[2 * p + s, 128 * hc:128 * (hc + 1), :],
                                qa[sl][:, hc, s, 3:3 + W],
                            ).then_inc(s_out[sl], 16)

                for pg in range(n_pg):
                    for sl in slots(pg):
                        load_pair(sl, pg)
                    for b in range(N_BLOCKS):
                        for k in range(N_TERMS):       # rounds 0..9
                            for sl in slots(pg):
                                ground(sl, pg, b, k)
                    for sl in slots(pg):
                        store_pair(sl, pg)
                for sl in range(2):
                    if n_pairs > sl:
                        pps = (n_pairs - 1 - sl) // 2 + 1
                        sync.wait_ge(s_out[sl], pps * 64)

            @block.tensor
            def _(tensor):
                def term_group(sl, pp, b, n):
                    # [mains_n] [fixup_n]
                    if n == 1:
                        if b == 0:
                            if pp == 0 and sl == 0:
                                tensor.wait_ge(s_w, 32)
                            tensor.wait_ge(s_in[sl], (pp + 1) * 64)
                        else:
                            tensor.wait_ge(s_act[sl], AT(pp, b - 1))
                    else:
                        tensor.wait_ge(s_dve[sl], D(pp, b, n - 1))
                        tensor.wait_ge(s_act[sl], A(pp, b, n - 1))
                    qs = qsrc(sl, n - 1)
                    for kc in range(KS):
                        for hc in range(2):
                            tensor.matmul(
                                prod[sl][hc][:], r(bands_sb[:, b, kc, :]),
                                r(qs[:, hc, :, kc:kc + W]),
                                start=(kc == 0), stop=False,
                            )
                    tensor.wait_ge(s_g[sl], GV(pp, b, n - 1))
                    for hc in range(2):
                        mm = tensor.matmul(
                            prod[sl][hc][:], r(fups_sb[:, b, hc, :]),
                            r(gt[sl][hc][:]),
                            start=False, stop=True,
                        )
                    mm.then_inc(s_pe[sl], 1)

                for pg in range(n_pg):
                    for b in range(N_BLOCKS):
                        for n in range(1, N_TERMS + 1):
                            for sl in slots(pg):
                                term_group(sl, pg, b, n)

            @block.vector
            def _(vector):
                vector.memset(qa[0][:].bitcast(F32), 0.0)
                vector.memset(qb[0][:].bitcast(F32), 0.0)
                vector.memset(qa[1][:].bitcast(F32), 0.0)
                vector.memset(qb[1][:].bitcast(F32), 0.0).then_inc(s_init, 1)
                for pg in range(n_pg):
                    for b in range(N_BLOCKS):
                        for n in range(1, N_TERMS + 1):
                            for sl in slots(pg):
                                vector.wait_ge(s_pe[sl], P(pg, b, n))
                                if n < N_TERMS:
                                    vector.tensor_scalar_mul(
                                        qsrc(sl, n)[:, 0, :, 3:3 + W],
                                        prod[sl][0][:], 1.0 / (n + 1),
                                    )
                                in1 = (qa[sl][:, 0, :, 3:3 + W].bitcast(F32)
                                       if n == 1 else resb[sl][:, 0, :, :])
                                vector.scalar_tensor_tensor(
                                    resb[sl][:, 0, :, :], prod[sl][0][:],
                                    1.0, in1, Mul, Add,
                                )
                                if n < N_TERMS:
                                    # ScalarE's chunk1 drain reads the same
                                    # PSUM bank; concurrent DVE+ACT access to
                                    # one bank is fatal -- serialize behind it.
                                    vector.wait_ge(s_act[sl], A(pg, b, n))
                                in1 = (qa[sl][:, 1, :, 3:3 + W].bitcast(F32)
                                       if n == 1 else resb[sl][:, 1, :, :])
                                vector.scalar_tensor_tensor(
                                    resb[sl][:, 1, :, :], prod[sl][1][:],
                                    1.0, in1, Mul, Add,
                                ).then_inc(s_dve[sl], 1)

            @block.scalar
            def _(scalar):
                for pg in range(n_pg):
                    for b in range(N_BLOCKS):
                        for n in range(1, N_TERMS):
                            for sl in slots(pg):
                                scalar.wait_ge(s_pe[sl], P(pg, b, n))
                                scalar.mul(
                                    qsrc(sl, n)[:, 1, :, 3:3 + W],
                                    prod[sl][1][:], 1.0 / (n + 1),
                                ).then_inc(s_act[sl], 1)
                        for sl in slots(pg):
                            scalar.wait_ge(s_dve[sl], D(pg, b, N_TERMS))
                            scalar.activation(qa[sl][:, 0, :, 3:3 + W],
                                              resb[sl][:, 0, :, :], Tanh)
                            scalar.activation(qa[sl][:, 1, :, 3:3 + W],
                                              resb[sl][:, 1, :, :], Tanh
                                              ).then_inc(s_act[sl], 1)

    return nc


_CACHE = {}


def _get_program(n_pairs):
    if n_pairs not in _CACHE:
        _CACHE[n_pairs] = _build_program(n_pairs)
    return _CACHE[n_pairs]


def run_on_hw(x, filters, trace=False, n_pairs=N_PAIRS, n_cores=N_CORES):
    """x: [B,1,H,W] fp32; returns (z [B,H,W], logdet [B,1], BassKernelResults)."""
    x = np.ascontiguousarray(np.asarray(x, np.float32))
    bands, fups = _host_matrices(np.asarray(filters, np.float32))
    nc = _get_program(n_pairs)
    per = 2 * n_pairs
    in_maps = []
    for c in range(n_cores):
        in_maps.append({
            "x": np.ascontiguousarray(x[c * per:(c + 1) * per, 0]),
            "bands": bands, "fups": fups,
        })
    kres = run_bass_kernel_spmd(nc, in_maps, list(range(n_cores)), trace=trace)
    z = np.concatenate([kres.results[c]["z_out"] for c in range(n_cores)], axis=0)

    mh = mw = (KS - 1) // 2
    tr = sum(float(np.trace(np.asarray(filters[i, :, :, mh, mw]))) for i in range(N_BLOCKS))
    ld = np.full((x.shape[0], 1), -np.float32(H * W) * np.float32(tr), np.float32)
    return z, ld, kres


def kernel(x, filters):
    z, ld, _ = run_on_hw(x, filters)
    return z, ld


if __name__ == "__main__":
    xs = np.load("/tmp/x2.npy")
    filt = np.load("/tmp/filters.npy")
    zr = np.load("/tmp/ref_z2.npy")
    xrep = np.tile(xs, (8, 1, 1, 1))
    z, ld, _ = run_on_hw(xrep, filt, n_pairs=1, n_cores=8)
    for c in range(8):
        rel = np.abs(z[2 * c:2 * c + 2] - zr).max() / np.abs(zr).max()
        print(f"core {c} rel err: {rel:.3e}")


# revision 29
# speedup vs baseline: 2392.9358x; 2392.9358x over previous
"""Trainium2 Bass kernel for nn_EquivariantConvExp.

Model: 4 blocks of { z = conv_exp(z, k_i); z = tanh(z) } on [B=256,1,256,256],
where conv_exp is the 10-term truncated power series of a 7x7 same-padded
single-channel conv. The log_det output is input-independent (host-computed).

Strategy: pure data parallel, 32 samples per core (16 pairs of 2 samples).
Per-sample conv term K@q is computed as banded matmuls along H:
  out[h_out, w] += Band_kc.T @ q[h_chunk, w + kc - 3]      (kc = 0..6)
with the image stored [h (2 chunks x 128 partitions), sample(2), 256+6 pad].
Cross-h-chunk band contributions come from 2 small fixup matmuls whose rhs is
a [21, 2, 256] gather (3 boundary rows x 7 w-shifts) built by SBUF->SBUF DMAs
with overlapping-window access patterns. The series sum accumulates in PSUM
via scaled-identity matmuls; the product chain is kept pre-divided
(q_n = prod_n/(n+1)) so accumulation is a plain matmul accumulate. Matmuls
run as float32r: full fp32 bits at bf16-rate for moving dim >= 256.

Raw bass (no Tile): this walrus build encodes at most ~1 semaphore wait per
instruction, so synchronization uses standalone wait_ge instructions with
hand-maintained cumulative counters per engine.
"""

import sys
from contextlib import ExitStack

import numpy as np

sys.path.insert(0, "/opt/trn_rl_repo")

import concourse.bass as bass
from concourse import mybir
from concourse.bass_utils import run_bass_kernel_spmd

F32 = mybir.dt.float32
F32R = mybir.dt.float32r
BF16 = mybir.dt.bfloat16
Tanh = mybir.ActivationFunctionType.Tanh

N_BLOCKS = 4
N_TERMS = 10
KS = 7
H = W = 256
N_CORES = 8
B = 256
PER_CORE = B // N_CORES          # 32
N_PAIRS = PER_CORE // 2          # 16
WP = W + 6                       # padded width 262
QF = 2 * WP                      # q free elems per hc: s(2) * 262
QROW = 2 * QF                    # q free elems per partition: hc*s*262


def _host_matrices(filters):
    """Banded / fixup / scaled-identity matrices, laid out for SBUF."""
    ks = [np.asarray(filters[fi, 0, 0], np.float32) for fi in reversed(range(N_BLOCKS))]
    bands = np.zeros((128, N_BLOCKS, KS, 128), np.float32)   # [p_in, blk, kc, p_out]
    fups = np.zeros((21, N_BLOCKS, 2, 128), np.float32)      # [(j*7+kc), blk, dir, m]
    for b, k in enumerate(ks):
        for kc in range(KS):
            for pi in range(128):
                for po in range(max(0, pi - 3), min(128, pi + 4)):
                    bands[pi, b, kc, po] = k[pi - po + 3, kc]
        for j in range(3):
            for kc in range(KS):
                rr = j * 7 + kc
                for m in range(128):
                    dh = (128 + j) - m            # dir 0 (down): h_in=128+j, h_out=m
                    if abs(dh) <= 3:
                        fups[rr, b, 0, m] = k[dh + 3, kc]
                    dh = (125 + j) - (128 + m)    # dir 1 (up): h_in=125+j, h_out=128+m
                    if abs(dh) <= 3:
                        fups[rr, b, 1, m] = k[dh + 3, kc]
    return bands, fups


def _build_program(n_pairs):
    nc = bass.Bass()
    x_in = nc.declare_dram_parameter("x", [2 * n_pairs, H, W], F32, isOutput=False)
    bands_in = nc.declare_dram_parameter("bands", [128, N_BLOCKS, KS, 128], BF16, isOutput=False)
    fups_in = nc.declare_dram_parameter("fups", [21, N_BLOCKS, 2, 128], BF16, isOutput=False)
    z_out = nc.declare_dram_parameter("z_out", [2 * n_pairs, H, W], F32, isOutput=True)

    r = lambda ap: ap
    ctx = ExitStack()
    sb = lambda name, shape, dt=BF16: ctx.enter_context(nc.sbuf_tensor(name, shape, dt))
    ps = lambda name, shape: ctx.enter_context(nc.psum_tensor(name, shape, F32))
    sem = lambda name: ctx.enter_context(nc.semaphore(name))
    Mul, Add = mybir.AluOpType.mult, mybir.AluOpType.add

    with ctx:
        bands_sb = sb("bands_sb", [128, N_BLOCKS, KS, 128], BF16)
        fups_sb = sb("fups_sb", [21, N_BLOCKS, 2, 128], BF16)
        qa = [sb(f"qa{sl}", [128, 2, 2, WP]) for sl in range(2)]
        qb = [sb(f"qb{sl}", [128, 2, 2, WP]) for sl in range(2)]
        gt = [[sb(f"g{sl}{d}", [21, 2, W]) for d in range(2)] for sl in range(2)]
        prod = [[ps(f"pr{sl}{hc}", [128, 2, W]) for hc in range(2)] for sl in range(2)]
        resb = [sb(f"res{sl}", [128, 2, 2, W], F32) for sl in range(2)]
        fout = [sb(f"fout{sl}", [128, 2, 2, W], F32) for sl in range(2)]
        fx = [sb(f"fx{sl}", [128, 2, 2, W], F32) for sl in range(2)]

        s_w = sem("s_w")
        s_init = sem("s_init")
        s_pe = [sem(f"s_pe{sl}") for sl in range(2)]
        s_dve = [sem(f"s_dve{sl}") for sl in range(2)]
        s_act = [sem(f"s_act{sl}") for sl in range(2)]
        s_g = [sem(f"s_g{sl}") for sl in range(2)]
        s_in = [sem(f"s_in{sl}") for sl in range(2)]
        s_cast = [sem(f"s_cast{sl}") for sl in range(2)]
        s_out = [sem(f"s_out{sl}") for sl in range(2)]

        def slots(pg):
            return [sl for sl in range(2) if 2 * pg + sl < n_pairs]

        def qsrc(sl, n):
            if n == 0:
                return qa[sl]
            return qb[sl] if n % 2 == 1 else qa[sl]

        # per-slot cumulative milestone formulas (pp = pair index on slot)
        NT, NB = N_TERMS, N_BLOCKS
        P = lambda pp, b, n: (pp * NB + b) * NT + n          # s_pe after fixup_n
        D = lambda pp, b, n: (pp * NB + b) * NT + n          # s_dve after pkg_n
        A = lambda pp, b, n: (pp * NB + b) * NT + n          # s_act after drain_n (n<NT)
        AT = lambda pp, b: (pp * NB + b) * NT + NT           # s_act after tanh
        GV = lambda pp, b, k: ((pp * NB + b) * NT + k + 1) * 64  # s_g after round k

        n_pg = (n_pairs + 1) // 2

        with nc.Block() as block:

            @block.sync
            def _(sync):
                sync.dma_start(bands_sb[:], bands_in[:]).then_inc(s_w, 16)
                sync.dma_start(fups_sb[:], fups_in[:]).then_inc(s_w, 16)

                def load_pair(sl, pg):
                    p = 2 * pg + sl
                    pp = pg
                    if pp == 0:
                        sync.wait_ge(s_init, 1)
                    else:
                        sync.wait_ge(s_out[sl], pp * 64)
                    for hc in range(2):
                        for s in range(2):
                            sync.dma_start(
                                fx[sl][:, hc, s, :],
                                x_in[2 * p + s, 128 * hc:128 * (hc + 1), :],
                            ).then_inc(s_in[sl], 16)

                def ground(sl, pp, b, k):
                    # gather for term k+1's fixups, reading q_k
                    qt = qsrc(sl, k)
                    t = qt[:].tensor
                    if k == 0:
                        if b == 0:
                            sync.wait_ge(s_cast[sl], pp + 1)
                        else:
                            sync.wait_ge(s_act[sl], AT(pp, b - 1))
                        # dir1 reads hc0 (chunk0): same source readiness
                    else:
                        sync.wait_ge(s_act[sl], A(pp, b, k))
                    for s in range(2):
                        src = bass.AP(t, QF + s * WP, [[QROW, 3], [1, KS], [1, W]])
                        sync.dma_start(gt[sl][0][:, s, :], src).then_inc(s_g[sl], 16)
                    if k != 0:
                        sync.wait_ge(s_dve[sl], D(pp, b, k))
                    for s in range(2):
                        src = bass.AP(t, 125 * QROW + s * WP, [[QROW, 3], [1, KS], [1, W]])
                        sync.dma_start(gt[sl][1][:, s, :], src).then_inc(s_g[sl], 16)

                def store_pair(sl, pg):
                    p = 2 * pg + sl
                    sync.wait_ge(s_act[sl], AT(pg, N_BLOCKS - 1))
                    for hc in range(2):
                        for s in range(2):
                            sync.dma_start(
                                z_out[2 * p + s, 128 * hc:128 * (hc + 1), :],
                                fout[sl][:, hc, s, :],
                            ).then_inc(s_out[sl], 16)

                for pg in range(n_pg):
                    for sl in slots(pg):
                        load_pair(sl, pg)
                    for b in range(N_BLOCKS):
                        for k in range(N_TERMS):       # rounds 0..9
                            for sl in slots(pg):
                                ground(sl, pg, b, k)
                    for sl in slots(pg):
                        store_pair(sl, pg)
                for sl in range(2):
                    if n_pairs > sl:
                        pps = (n_pairs - 1 - sl) // 2 + 1
                        sync.wait_ge(s_out[sl], pps * 64)

            @block.tensor
            def _(tensor):
                def term_group(sl, pp, b, n):
                    # [mains_n] [fixup_n]
                    if n == 1:
                        if b == 0:
                            if pp == 0 and sl == 0:
                                tensor.wait_ge(s_w, 32)
                            tensor.wait_ge(s_cast[sl], pp + 1)
                        else:
                            tensor.wait_ge(s_act[sl], AT(pp, b - 1))
                    else:
                        tensor.wait_ge(s_dve[sl], D(pp, b, n - 1))
                        tensor.wait_ge(s_act[sl], A(pp, b, n - 1))
                    qs = qsrc(sl, n - 1)
                    for kc in range(KS):
                        for hc in range(2):
                            tensor.matmul(
                                prod[sl][hc][:], r(bands_sb[:, b, kc, :]),
                                r(qs[:, hc, :, kc:kc + W]),
                                start=(kc == 0), stop=False,
                            )
                    tensor.wait_ge(s_g[sl], GV(pp, b, n - 1))
                    for hc in range(2):
                        mm = tensor.matmul(
                            prod[sl][hc][:], r(fups_sb[:, b, hc, :]),
                            r(gt[sl][hc][:]),
                            start=False, stop=True,
                        )
                    mm.then_inc(s_pe[sl], 1)

                for pg in range(n_pg):
                    for b in range(N_BLOCKS):
                        for n in range(1, N_TERMS + 1):
                            for sl in slots(pg):
                                term_group(sl, pg, b, n)

            @block.vector
            def _(vector):
                vector.memset(qa[0][:], 0.0)
                vector.memset(qb[0][:], 0.0)
                vector.memset(qa[1][:], 0.0)
                vector.memset(qb[1][:], 0.0).then_inc(s_init, 1)
                for pg in range(n_pg):
                    for sl in slots(pg):
                        vector.wait_ge(s_in[sl], (pg + 1) * 64)
                        vector.tensor_copy(qa[sl][:, 0, :, 3:3 + W], fx[sl][:, 0, :, :])
                        vector.tensor_copy(qa[sl][:, 1, :, 3:3 + W], fx[sl][:, 1, :, :]
                                           ).then_inc(s_cast[sl], 1)
                    for b in range(N_BLOCKS):
                        for n in range(1, N_TERMS + 1):
                            for sl in slots(pg):
                                vector.wait_ge(s_pe[sl], P(pg, b, n))
                                if n < N_TERMS:
                                    vector.tensor_scalar_mul(
                                        qsrc(sl, n)[:, 0, :, 3:3 + W],
                                        prod[sl][0][:], 1.0 / (n + 1),
                                    )
                                in1 = (fx[sl][:, 0, :, :]
                                       if n == 1 else resb[sl][:, 0, :, :])
                                vector.scalar_tensor_tensor(
                                    resb[sl][:, 0, :, :], prod[sl][0][:],
                                    1.0, in1, Mul, Add,
                                )
                                if n < N_TERMS:
                                    # ScalarE's chunk1 drain reads the same
                                    # PSUM bank; concurrent DVE+ACT access to
                                    # one bank is fatal -- serialize behind it.
                                    vector.wait_ge(s_act[sl], A(pg, b, n))
                                in1 = (fx[sl][:, 1, :, :]
                                       if n == 1 else resb[sl][:, 1, :, :])
                                vector.scalar_tensor_tensor(
                                    resb[sl][:, 1, :, :], prod[sl][1][:],
                                    1.0, in1, Mul, Add,
                                ).then_inc(s_dve[sl], 1)

            @block.scalar
            def _(scalar):
                for pg in range(n_pg):
                    for b in range(N_BLOCKS):
                        for n in range(1, N_TERMS):
                            for sl in slots(pg):
                                scalar.wait_ge(s_pe[sl], P(pg, b, n))
                                scalar.mul(
                                    qsrc(sl, n)[:, 1, :, 3:3 + W],
                                    prod[sl][1][:], 1.0 / (n + 1),
                                ).then_inc(s_act[sl], 1)
                        for sl in slots(pg):
                            scalar.wait_ge(s_dve[sl], D(pg, b, N_TERMS))
                            if b < N_BLOCKS - 1:
                                scalar.activation(fx[sl][:, 0, :, :],
                                                  resb[sl][:, 0, :, :], Tanh)
                                scalar.activation(fx[sl][:, 1, :, :],
                                                  resb[sl][:, 1, :, :], Tanh)
                                scalar.activation(qa[sl][:, 0, :, 3:3 + W],
                                                  resb[sl][:, 0, :, :], Tanh)
                                scalar.activation(qa[sl][:, 1, :, 3:3 + W],
                                                  resb[sl][:, 1, :, :], Tanh
                                                  ).then_inc(s_act[sl], 1)
                            else:
                                scalar.activation(fout[sl][:, 0, :, :],
                                                  resb[sl][:, 0, :, :], Tanh)
                                scalar.activation(fout[sl][:, 1, :, :],
                                                  resb[sl][:, 1, :, :], Tanh
                                                  ).then_inc(s_act[sl], 1)

    return nc


_CACHE = {}


def _get_program(n_pairs):
    if n_pairs not in _CACHE:
        _CACHE[n_pairs] = _build_program(n_pairs)
    return _CACHE[n_pairs]


def run_on_hw(x, filters, trace=False, n_pairs=N_PAIRS, n_cores=N_CORES):
    """x: [B,1,H,W] fp32; returns (z [B,H,W], logdet [B,1], BassKernelResults)."""
    import ml_dtypes
    x = np.ascontiguousarray(np.asarray(x, np.float32))
    bands, fups = _host_matrices(np.asarray(filters, np.float32))
    bands = bands.astype(ml_dtypes.bfloat16)
    fups = fups.astype(ml_dtypes.bfloat16)
    nc = _get_program(n_pairs)
    per = 2 * n_pairs
    in_maps = []
    for c in range(n_cores):
        in_maps.append({
            "x": np.ascontiguousarray(x[c * per:(c + 1) * per, 0]),
            "bands": bands, "fups": fups,
        })
    kres = run_bass_kernel_spmd(nc, in_maps, list(range(n_cores)), trace=trace)
    z = np.concatenate([kres.results[c]["z_out"] for c in range(n_cores)], axis=0)

    mh = mw = (KS - 1) // 2
    tr = sum(float(np.trace(np.asarray(filters[i, :, :, mh, mw]))) for i in range(N_BLOCKS))
    ld = np.full((x.shape[0], 1), -np.float32(H * W) * np.float32(tr), np.float32)
    return z, ld, kres


def kernel(x, filters):
    z, ld, _ = run_on_hw(x, filters)
    return z, ld


if __name__ == "__main__":
    xs = np.load("/tmp/x2.npy")
    filt = np.load("/tmp/filters.npy")
    zr = np.load("/tmp/ref_z2.npy")
    xrep = np.tile(xs, (8, 1, 1, 1))
    z, ld, _ = run_on_hw(xrep, filt, n_pairs=1, n_cores=8)
    for c in range(8):
        rel = np.abs(z[2 * c:2 * c + 2] - zr).max() / np.abs(zr).max()
        print(f"core {c} rel err: {rel:.3e}")


# revision 31
# speedup vs baseline: 2525.0024x; 1.0552x over previous
"""Trainium2 Bass kernel for nn_EquivariantConvExp.

Model: 4 blocks of { z = conv_exp(z, k_i); z = tanh(z) } on [B=256,1,256,256],
where conv_exp is the 10-term truncated power series of a 7x7 same-padded
single-channel conv. The log_det output is input-independent (host-computed).

Strategy: pure data parallel, 32 samples per core (16 pairs of 2 samples).
Per-sample conv term K@q is computed as banded matmuls along H:
  out[h_out, w] += Band_kc.T @ q[h_chunk, w + kc - 3]      (kc = 0..6)
with the image stored [h (2 chunks x 128 partitions), sample(2), 256+6 pad].
Cross-h-chunk band contributions come from 2 small fixup matmuls whose rhs is
a [21, 2, 256] gather (3 boundary rows x 7 w-shifts) built by SBUF->SBUF DMAs
with overlapping-window access patterns. The series sum accumulates in PSUM
via scaled-identity matmuls; the product chain is kept pre-divided
(q_n = prod_n/(n+1)); the series sum accumulates in an fp32 SBUF tensor via
VectorE scalar_tensor_tensor ops reading PSUM. The conv chain runs in bf16
(weights + activations; fp32 PSUM accumulation); the base signal of each
block (input x / tanh output) additionally stays in fp32 through the result
path, roughly halving the end-to-end error (~6e-3 max-rel vs fp32 reference).

Raw bass (no Tile): this walrus build encodes at most ~1 semaphore wait per
instruction, so synchronization uses standalone wait_ge instructions with
hand-maintained cumulative counters per engine.
"""

import sys
from contextlib import ExitStack

import numpy as np

sys.path.insert(0, "/opt/trn_rl_repo")

import concourse.bass as bass
from concourse import mybir
from concourse.bass_utils import run_bass_kernel_spmd

F32 = mybir.dt.float32
F32R = mybir.dt.float32r
BF16 = mybir.dt.bfloat16
Tanh = mybir.ActivationFunctionType.Tanh

N_BLOCKS = 4
N_TERMS = 10
KS = 7
H = W = 256
N_CORES = 8
B = 256
PER_CORE = B // N_CORES          # 32
N_PAIRS = PER_CORE // 2          # 16
NSL = 4                          # sample-pairs in flight (PSUM: NSL*2 banks)
WP = W + 6                       # padded width 262
QF = 2 * WP                      # q free elems per hc: s(2) * 262
QROW = 2 * QF                    # q free elems per partition: hc*s*262


def _host_matrices(filters):
    """Banded / fixup / scaled-identity matrices, laid out for SBUF."""
    ks = [np.asarray(filters[fi, 0, 0], np.float32) for fi in reversed(range(N_BLOCKS))]
    bands = np.zeros((128, N_BLOCKS, KS, 128), np.float32)   # [p_in, blk, kc, p_out]
    fups = np.zeros((21, N_BLOCKS, 2, 128), np.float32)      # [(j*7+kc), blk, dir, m]
    for b, k in enumerate(ks):
        for kc in range(KS):
            for pi in range(128):
                for po in range(max(0, pi - 3), min(128, pi + 4)):
                    bands[pi, b, kc, po] = k[pi - po + 3, kc]
        for j in range(3):
            for kc in range(KS):
                rr = j * 7 + kc
                for m in range(128):
                    dh = (128 + j) - m            # dir 0 (down): h_in=128+j, h_out=m
                    if abs(dh) <= 3:
                        fups[rr, b, 0, m] = k[dh + 3, kc]
                    dh = (125 + j) - (128 + m)    # dir 1 (up): h_in=125+j, h_out=128+m
                    if abs(dh) <= 3:
                        fups[rr, b, 1, m] = k[dh + 3, kc]
    return bands, fups


def _build_program(n_pairs):
    nc = bass.Bass()
    x_in = nc.declare_dram_parameter("x", [2 * n_pairs, H, W], F32, isOutput=False)
    bands_in = nc.declare_dram_parameter("bands", [128, N_BLOCKS, KS, 128], BF16, isOutput=False)
    fups_in = nc.declare_dram_parameter("fups", [21, N_BLOCKS, 2, 128], BF16, isOutput=False)
    z_out = nc.declare_dram_parameter("z_out", [2 * n_pairs, H, W], F32, isOutput=True)

    r = lambda ap: ap
    ctx = ExitStack()
    sb = lambda name, shape, dt=BF16: ctx.enter_context(nc.sbuf_tensor(name, shape, dt))
    ps = lambda name, shape: ctx.enter_context(nc.psum_tensor(name, shape, F32))
    sem = lambda name: ctx.enter_context(nc.semaphore(name))
    Mul, Add = mybir.AluOpType.mult, mybir.AluOpType.add

    with ctx:
        bands_sb = sb("bands_sb", [128, N_BLOCKS, KS, 128], BF16)
        fups_sb = sb("fups_sb", [21, N_BLOCKS, 2, 128], BF16)
        qa = [sb(f"qa{sl}", [128, 2, 2, WP]) for sl in range(NSL)]
        qb = [sb(f"qb{sl}", [128, 2, 2, WP]) for sl in range(NSL)]
        gt = [[sb(f"g{sl}{d}", [21, 2, W]) for d in range(2)] for sl in range(NSL)]
        prod = [[ps(f"pr{sl}{hc}", [128, 2, W]) for hc in range(2)] for sl in range(NSL)]
        resb = [sb(f"res{sl}", [128, 2, 2, W], F32) for sl in range(NSL)]
        fout = [sb(f"fout{sl}", [128, 2, 2, W], F32) for sl in range(NSL)]
        fx = [sb(f"fx{sl}", [128, 2, 2, W], F32) for sl in range(NSL)]

        s_w = sem("s_w")
        s_init = sem("s_init")
        s_pe = [sem(f"s_pe{sl}") for sl in range(NSL)]
        s_dve = [sem(f"s_dve{sl}") for sl in range(NSL)]
        s_act = [sem(f"s_act{sl}") for sl in range(NSL)]
        s_g = [sem(f"s_g{sl}") for sl in range(NSL)]
        s_in = [sem(f"s_in{sl}") for sl in range(NSL)]
        s_cast = [sem(f"s_cast{sl}") for sl in range(NSL)]
        s_out = [sem(f"s_out{sl}") for sl in range(NSL)]

        def slots(pg):
            return [sl for sl in range(NSL) if NSL * pg + sl < n_pairs]

        def qsrc(sl, n):
            if n == 0:
                return qa[sl]
            return qb[sl] if n % 2 == 1 else qa[sl]

        # per-slot cumulative milestone formulas (pp = pair index on slot)
        NT, NB = N_TERMS, N_BLOCKS
        P = lambda pp, b, n: (pp * NB + b) * NT + n          # s_pe after fixup_n
        D = lambda pp, b, n: (pp * NB + b) * NT + n          # s_dve after pkg_n
        A = lambda pp, b, n: (pp * NB + b) * NT + n          # s_act after drain_n (n<NT)
        AT = lambda pp, b: (pp * NB + b) * NT + NT           # s_act after tanh
        GV = lambda pp, b, k: ((pp * NB + b) * NT + k + 1) * 64  # s_g after round k

        n_pg = (n_pairs + NSL - 1) // NSL

        with nc.Block() as block:

            @block.sync
            def _(sync):
                sync.dma_start(bands_sb[:], bands_in[:]).then_inc(s_w, 16)
                sync.dma_start(fups_sb[:], fups_in[:]).then_inc(s_w, 16)

                def load_pair(sl, pg):
                    p = NSL * pg + sl
                    pp = pg
                    if pp == 0:
                        sync.wait_ge(s_init, 1)
                    else:
                        sync.wait_ge(s_out[sl], pp * 64)
                    for hc in range(2):
                        for s in range(2):
                            sync.dma_start(
                                fx[sl][:, hc, s, :],
                                x_in[2 * p + s, 128 * hc:128 * (hc + 1), :],
                            ).then_inc(s_in[sl], 16)

                def ground(sl, pp, b, k):
                    # gather for term k+1's fixups, reading q_k
                    qt = qsrc(sl, k)
                    t = qt[:].tensor
                    if k == 0:
                        if b == 0:
                            sync.wait_ge(s_cast[sl], pp + 1)
                        else:
                            sync.wait_ge(s_act[sl], AT(pp, b - 1))
                        # dir1 reads hc0 (chunk0): same source readiness
                    else:
                        sync.wait_ge(s_act[sl], A(pp, b, k))
                    for s in range(2):
                        src = bass.AP(t, QF + s * WP, [[QROW, 3], [1, KS], [1, W]])
                        sync.dma_start(gt[sl][0][:, s, :], src).then_inc(s_g[sl], 16)
                    if k != 0:
                        sync.wait_ge(s_dve[sl], D(pp, b, k))
                    for s in range(2):
                        src = bass.AP(t, 125 * QROW + s * WP, [[QROW, 3], [1, KS], [1, W]])
                        sync.dma_start(gt[sl][1][:, s, :], src).then_inc(s_g[sl], 16)

                def store_pair(sl, pg):
                    p = NSL * pg + sl
                    sync.wait_ge(s_act[sl], AT(pg, N_BLOCKS - 1))
                    for hc in range(2):
                        for s in range(2):
                            sync.dma_start(
                                z_out[2 * p + s, 128 * hc:128 * (hc + 1), :],
                                fout[sl][:, hc, s, :],
                            ).then_inc(s_out[sl], 16)

                for pg in range(n_pg):
                    for sl in slots(pg):
                        load_pair(sl, pg)
                    for b in range(N_BLOCKS):
                        for k in range(N_TERMS):       # rounds 0..9
                            for sl in slots(pg):
                                ground(sl, pg, b, k)
                    for sl in slots(pg):
                        store_pair(sl, pg)
                for sl in range(NSL):
                    if n_pairs > sl:
                        pps = (n_pairs - 1 - sl) // NSL + 1
                        sync.wait_ge(s_out[sl], pps * 64)

            @block.tensor
            def _(tensor):
                def term_group(sl, pp, b, n):
                    # [mains_n] [fixup_n]
                    if n == 1:
                        if b == 0:
                            if pp == 0 and sl == 0:
                                tensor.wait_ge(s_w, 32)
                            tensor.wait_ge(s_cast[sl], pp + 1)
                        else:
                            tensor.wait_ge(s_act[sl], AT(pp, b - 1))
                    else:
                        tensor.wait_ge(s_dve[sl], D(pp, b, n - 1))
                        tensor.wait_ge(s_act[sl], A(pp, b, n - 1))
                    qs = qsrc(sl, n - 1)
                    for kc in range(KS):
                        for hc in range(2):
                            tensor.matmul(
                                prod[sl][hc][:], r(bands_sb[:, b, kc, :]),
                                r(qs[:, hc, :, kc:kc + W]),
                                start=(kc == 0), stop=False,
                            )
                    tensor.wait_ge(s_g[sl], GV(pp, b, n - 1))
                    for hc in range(2):
                        mm = tensor.matmul(
                            prod[sl][hc][:], r(fups_sb[:, b, hc, :]),
                            r(gt[sl][hc][:]),
                            start=False, stop=True,
                        )
                    mm.then_inc(s_pe[sl], 1)

                for pg in range(n_pg):
                    for b in range(N_BLOCKS):
                        for n in range(1, N_TERMS + 1):
                            for sl in slots(pg):
                                term_group(sl, pg, b, n)

            @block.vector
            def _(vector):
                for sl in range(NSL):
                    vector.memset(qa[sl][:], 0.0)
                    mm = vector.memset(qb[sl][:], 0.0)
                mm.then_inc(s_init, 1)
                for pg in range(n_pg):
                    for sl in slots(pg):
                        vector.wait_ge(s_in[sl], (pg + 1) * 64)
                        vector.tensor_copy(qa[sl][:, 0, :, 3:3 + W], fx[sl][:, 0, :, :])
                        vector.tensor_copy(qa[sl][:, 1, :, 3:3 + W], fx[sl][:, 1, :, :]
                                           ).then_inc(s_cast[sl], 1)
                    for b in range(N_BLOCKS):
                        for n in range(1, N_TERMS + 1):
                            for sl in slots(pg):
                                vector.wait_ge(s_pe[sl], P(pg, b, n))
                                if n < N_TERMS:
                                    vector.tensor_scalar_mul(
                                        qsrc(sl, n)[:, 0, :, 3:3 + W],
                                        prod[sl][0][:], 1.0 / (n + 1),
                                    )
                                in1 = (fx[sl][:, 0, :, :]
                                       if n == 1 else resb[sl][:, 0, :, :])
                                vector.scalar_tensor_tensor(
                                    resb[sl][:, 0, :, :], prod[sl][0][:],
                                    1.0, in1, Mul, Add,
                                )
                                if n < N_TERMS:
                                    # ScalarE's chunk1 drain reads the same
                                    # PSUM bank; concurrent DVE+ACT access to
                                    # one bank is fatal -- serialize behind it.
                                    vector.wait_ge(s_act[sl], A(pg, b, n))
                                in1 = (fx[sl][:, 1, :, :]
                                       if n == 1 else resb[sl][:, 1, :, :])
                                vector.scalar_tensor_tensor(
                                    resb[sl][:, 1, :, :], prod[sl][1][:],
                                    1.0, in1, Mul, Add,
                                ).then_inc(s_dve[sl], 1)

            @block.scalar
            def _(scalar):
                for pg in range(n_pg):
                    for b in range(N_BLOCKS):
                        for n in range(1, N_TERMS):
                            for sl in slots(pg):
                                scalar.wait_ge(s_pe[sl], P(pg, b, n))
                                scalar.mul(
                                    qsrc(sl, n)[:, 1, :, 3:3 + W],
                                    prod[sl][1][:], 1.0 / (n + 1),
                                ).then_inc(s_act[sl], 1)
                        for sl in slots(pg):
                            scalar.wait_ge(s_dve[sl], D(pg, b, N_TERMS))
                            if b < N_BLOCKS - 1:
                                scalar.activation(fx[sl][:, 0, :, :],
                                                  resb[sl][:, 0, :, :], Tanh)
                                scalar.activation(fx[sl][:, 1, :, :],
                                                  resb[sl][:, 1, :, :], Tanh)
                                scalar.activation(qa[sl][:, 0, :, 3:3 + W],
                                                  resb[sl][:, 0, :, :], Tanh)
                                scalar.activation(qa[sl][:, 1, :, 3:3 + W],
                                                  resb[sl][:, 1, :, :], Tanh
                                                  ).then_inc(s_act[sl], 1)
                            else:
                                scalar.activation(fout[sl][:, 0, :, :],
                                                  resb[sl][:, 0, :, :], Tanh)
                                scalar.activation(fout[sl][:, 1, :, :],
                                                  resb[sl][:, 1, :, :], Tanh
                                                  ).then_inc(s_act[sl], 1)

    return nc


_CACHE = {}


def _get_program(n_pairs):
    if n_pairs not in _CACHE:
        _CACHE[n_pairs] = _build_program(n_pairs)
    return _CACHE[n_pairs]


def run_on_hw(x, filters, trace=False, n_pairs=N_PAIRS, n_cores=N_CORES):
    """x: [B,1,H,W] fp32; returns (z [B,H,W], logdet [B,1], BassKernelResults)."""
    import ml_dtypes
    x = np.ascontiguousarray(np.asarray(x, np.float32))
    bands, fups = _host_matrices(np.asarray(filters, np.float32))
    bands = bands.astype(ml_dtypes.bfloat16)
    fups = fups.astype(ml_dtypes.bfloat16)
    nc = _get_program(n_pairs)
    per = 2 * n_pairs
    in_maps = []
    for c in range(n_cores):
        in_maps.append({
            "x": np.ascontiguousarray(x[c * per:(c + 1) * per, 0]),
            "bands": bands, "fups": fups,
        })
    kres = run_bass_kernel_spmd(nc, in_maps, list(range(n_cores)), trace=trace)
    z = np.concatenate([kres.results[c]["z_out"] for c in range(n_cores)], axis=0)

    mh = mw = (KS - 1) // 2
    tr = sum(float(np.trace(np.asarray(filters[i, :, :, mh, mw]))) for i in range(N_BLOCKS))
    ld = np.full((x.shape[0], 1), -np.float32(H * W) * np.float32(tr), np.float32)
    return z, ld, kres


def kernel(x, filters):
    z, ld, _ = run_on_hw(x, filters)
    return z, ld


if __name__ == "__main__":
    xs = np.load("/tmp/x2.npy")
    filt = np.load("/tmp/filters.npy")
    zr = np.load("/tmp/ref_z2.npy")
    xrep = np.tile(xs, (8, 1, 1, 1))
    z, ld, _ = run_on_hw(xrep, filt, n_pairs=1, n_cores=8)
    for c in range(8):
        rel = np.abs(z[2 * c:2 * c + 2] - zr).max() / np.abs(zr).max()
        print(f"core {c} rel err: {rel:.3e}")
